# revision 22
# baseline (speedup 1.0000x reference)
"""Masked-BCE valid-region loss on 8 Trainium2 NeuronCores.

Inputs (full): cancer_logits [32,1,512,512] f32, label [32] f32,
prostate_mask [32,1,512,512] f32, needle_mask [32,1,512,512] f32.
Output: scalar f32 loss.

Sharding: data-parallel over batch - 4 images per core. Per-core HBM
streams: masks as ONE chunk-blocked [p|n] fp8e4m3 stream (2 MB) and
logits as bf16 (2 MB) - 4 MB/core vs 12 MB f32. Mask threshold flips
from fp8 rounding hit numerator and denominator on the same pixels,
so the loss ratio moves only ~1e-4.

Math: with m = (min(p,n) > 0.5) and y constant per image,

    bce = softplus(x) - x*y
    sum_masked softplus(x) = sum ln(1 + e^xm) - (N - count)*ln2

since masked-out elements have xm = 0 and contribute ln2. The ln of
1M elements is folded as ln(prod) over groups of 4 via a bf16 multiply
tree ((1+e)^4 <= 2.4e11, far below bf16 overflow), so ACT runs one
full-size pass (exp) plus a quarter-size ln per chunk - both live in
the natural_log_exp_and_others table set, so there are no mid-kernel
ACT table switches and the lns pipeline with later chunks' exps.

Two custom DVE ops (registered into dve_ops at import, table bytes are
embedded in the HLO at trace time):

    MASK_MIN_GT_CNT_ANT: out = (min(p8,n8) > 0.5), accum = count.
        Custom ops run at 1x for any dtype, so the masks can stay fp8
        (halving their DMA cost) while min+compare+count fuse into one
        pass - stock ops would need a bf16 cast-DMA (write-side bound),
        two DVE passes, and a TensorE count reduction.
    TREE_MUL_P1_ANT: out = (a+1)*(b+1) - folds the exp+1 into the
        first tree level.

Device pipeline per chunk:

    m   = (min(p8,n8) > 0.5), count     # custom DVE, fp8 in
    xm  = m * x                         # DVE tensor_tensor, 2x bf16
    e   = exp(xm)                       # ACT full-size pass
    w2  = (e_lo+1)*(e_hi+1)             # custom DVE
    w4  = w2_lo * w2_hi                 # DVE 2x
    ln(w4) accum -> lncols[:, c]        # ACT quarter-size, interleaved
    sxm_img += ones' @ xm               # TensorE -> PSUM (evacuated
                                        #  as each image completes)

tensor_scalar cannot carry an accumulator (BIR verifier rejects it)
and Pool/GpSimd cannot run TensorScalarPtr at all, which is why the
per-image sum(x*m) rides TensorE ones-matmuls. The two output DMAs go
out on different HWDGE engines (sync + scalar) to overlap completion.
"""

import sys

for _p in ("/opt/trn_rl_repo", "/root/.axon_site/_ro/trn_rl_repo"):
    if _p not in sys.path:
        sys.path.append(_p)

import ml_dtypes
import numpy as np

import concourse.bacc as bacc
import concourse.tile as tile
from concourse import mybir
from concourse.bass_utils import run_bass_kernel_spmd

B, H, W = 32, 512, 512
N_CORES = 8
IMGS_PER_CORE = B // N_CORES  # 4
P = 128
FD = (H * W) // P  # 2048 free-dim elements per partition per image
N_PER_IMG = H * W  # 262144
TOT_FD = IMGS_PER_CORE * FD  # 8192
# chunks: multiples of 512 (PE blocks); small first chunk for ramp,
# small last chunk for a short serial tail.
CHUNK_FDS = [1024, 1536, 2048, 1536, 1536, 512]
N_CHUNKS = len(CHUNK_FDS)

_nc_cache = None


def _patch_act_tables():
    """Pin every activation to natural_log_exp_and_others (exp + ln) so
    exactly one ACT_TABLE_LOAD is emitted."""
    import concourse.hw_specs as hw_specs

    if getattr(bacc, "_act_tables_patched", False):
        return
    orig = hw_specs.get_activation_tables

    def patched(module_arch):
        tables = orig(module_arch)
        keep = "natural_log_exp_and_others"
        if keep in tables:
            tables = {
                name: (funcs if name == keep else set())
                for name, funcs in tables.items()
            }
        return tables

    bacc.get_activation_tables = patched
    bacc._act_tables_patched = True


def _register_custom_ops():
    """Register the two fused DVE ops into dve_ops' tables. The uops_sha
    pin is computed from lower() here, so it is self-consistent by
    construction; correctness is asserted against numpy by the test."""
    import concourse.dve_ops as dvo
    from concourse.dve_spec import AluOp, C0, One, Spec, Src0, Src1, lower, minn
    from concourse.dve_uop import DveOpSpec

    if hasattr(dvo, "MASK_MIN_GT_CNT_ANT"):
        return

    def mask_ref(in0, in1, c0, c1, c2):
        m = (np.minimum(in0, in1) > c0).astype(np.float32)
        return m, m.sum(axis=1, keepdims=True)

    specs = [
        (
            "MASK_MIN_GT_CNT_ANT",
            Spec(body=(minn(Src0, Src1) > C0), accum=AluOp.ADD, reference=mask_ref),
        ),
        (
            "TREE_MUL_P1_ANT",
            Spec(
                body=(Src0 + One) * (Src1 + One),
                reference=lambda in0, in1, c0, c1, c2: (in0 + 1.0) * (in1 + 1.0),
            ),
        ),
    ]
    for name, spec in specs:
        row = dvo._CUSTOM_DVE_ROW_BASE + len(dvo.OPS)
        shas = {}
        for ver in ("v3", "v4"):
            s = DveOpSpec(name=name, opcode=row, uops=lower(spec, ver=ver), rd1_en=True)
            shas[ver] = s.sha(ver)
        op = dvo.DveOp(name, spec, subdim=False, uops_sha=shas)
        dvo.OPS.append(op)
        dvo._SUB_OPCODE_FOR_NAME[name] = row
        dvo.CUSTOM_DVE_SPECS[name] = spec
        setattr(dvo, name, op)


def _build_bass():
    _patch_act_tables()
    _register_custom_ops()
    import concourse.dve_ops as dvo

    f32 = mybir.dt.float32
    bf16 = mybir.dt.bfloat16
    fp8 = mybir.dt.float8e4
    nc = bacc.Bacc()
    u8 = mybir.dt.uint8
    # one byte stream per core: per chunk [p fp8 | n fp8 | x bf16-bytes]
    pnx_d = nc.dram_tensor("pnx8", [P, 4 * TOT_FD], u8, kind="ExternalInput")
    # [count col per chunk | ln col per chunk]
    cols_o = nc.dram_tensor("cols", [P, 2 * N_CHUNKS], f32, kind="ExternalOutput")
    # img0..img3 rows, 512 f32 each
    red_o = nc.dram_tensor("red", [1, 4 * 512], f32, kind="ExternalOutput")

    with tile.TileContext(nc) as tc:
        with (
            tc.tile_pool(name="io", bufs=3) as io_pool,
            tc.tile_pool(name="work", bufs=3) as work_pool,
            tc.tile_pool(name="keep", bufs=1) as keep_pool,
            tc.tile_pool(name="psum", bufs=1, space="PSUM") as psum_pool,
        ):
            ones1 = keep_pool.tile([P, 1], bf16)
            nc.gpsimd.memset(ones1, 1.0)
            cols = keep_pool.tile([P, 2 * N_CHUNKS], f32)
            red_sb = keep_pool.tile([1, 4 * 512], f32)
            img_ps = [
                psum_pool.tile([1, 512], f32, tag=f"img{i}", name=f"img_ps{i}")
                for i in range(IMGS_PER_CORE)
            ]

            blocks_per_img = FD // 512
            img_done = [0] * IMGS_PER_CORE
            off = 0
            for c, cfd in enumerate(CHUNK_FDS):
                raw = io_pool.tile([P, 4 * cfd], u8, tag="raw")
                nc.sync.dma_start(out=raw, in_=pnx_d[:, 4 * off : 4 * off + 4 * cfd])
                p8v = raw[:, 0:cfd].bitcast(fp8)
                n8v = raw[:, cfd : 2 * cfd].bitcast(fp8)
                xb = raw[:, 2 * cfd : 4 * cfd].bitcast(bf16)

                # m = (min(p, n) > 0.5) with fused count accumulation
                m = work_pool.tile([P, cfd], bf16, tag="m")
                nc.vector._custom_dve(
                    dvo.MASK_MIN_GT_CNT_ANT,
                    out=m, in0=p8v, in1=n8v,
                    s0=0.5, accum_out=cols[:, c : c + 1],
                )
                xm = work_pool.tile([P, cfd], bf16, tag="xm")
                nc.vector.tensor_tensor(
                    out=xm, in0=m, in1=xb, op=mybir.AluOpType.mult
                )
                # e = exp(xm); tree folds the +1 into its first level
                e = work_pool.tile([P, cfd], bf16, tag="e")
                nc.scalar.activation(
                    out=e, in_=xm, func=mybir.ActivationFunctionType.Exp,
                )
                h = cfd // 2
                w2 = work_pool.tile([P, h], bf16, tag="w2")
                nc.vector._custom_dve(
                    dvo.TREE_MUL_P1_ANT, out=w2, in0=e[:, :h], in1=e[:, h:]
                )
                # ln of the chunk's pair-products, accumulated per
                # partition; exp and ln share one table set so this
                # pipelines freely. ACT has slack, so a second tree level
                # on the (bottleneck) DVE would cost more than the wider ln.
                lnv = work_pool.tile([P, h], bf16, tag="lnv")
                nc.scalar.activation(
                    out=lnv, in_=w2, func=mybir.ActivationFunctionType.Ln,
                    accum_out=cols[:, N_CHUNKS + c : N_CHUNKS + c + 1],
                )

                # TensorE: per-image sum(xm)
                for s0 in range(0, cfd, 512):
                    i = (off + s0) // FD
                    nc.tensor.matmul(
                        img_ps[i], ones1, xm[:, s0 : s0 + 512],
                        start=(img_done[i] == 0),
                        stop=(img_done[i] == blocks_per_img - 1),
                    )
                    img_done[i] += 1
                    if img_done[i] == blocks_per_img:
                        # evacuate this image's PSUM bank while later
                        # chunks still stream (keeps it off the tail);
                        # ACT sits closest to PSUM and spares the DVE
                        # (GpSimd cannot access PSUM at all)
                        if i == IMGS_PER_CORE - 1:
                            # the last image's copy would sit on the ACT
                            # tail behind the final ln -> then-idle DVE
                            nc.vector.tensor_scalar_add(
                                out=red_sb[:, i * 512 : (i + 1) * 512],
                                in0=img_ps[i], scalar1=0.0,
                            )
                        else:
                            nc.scalar.copy(
                                out=red_sb[:, i * 512 : (i + 1) * 512],
                                in_=img_ps[i],
                            )
                off += cfd

            # parallel completion: cols via the scalar HWDGE ring, red via sync
            nc.scalar.dma_start(out=cols_o[:], in_=cols)
            nc.sync.dma_start(out=red_o[:], in_=red_sb)
    nc.finalize()
    return nc


def _get_nc():
    global _nc_cache
    if _nc_cache is None:
        _nc_cache = _build_bass()
    return _nc_cache


# global col offsets of each chunk
_CHUNK_OFFS = []
_off = 0
for _cfd in CHUNK_FDS:
    _CHUNK_OFFS.append(_off)
    _off += _cfd


def _make_in_maps(cancer_logits, prostate_mask, needle_mask):
    bf = ml_dtypes.bfloat16
    f8 = ml_dtypes.float8_e4m3

    # [B,1,H,W] -> [CORE, P, IMG*FD] image-major flat per-partition streams
    def pack(a, dt):
        a = np.asarray(a, dtype=np.float32).reshape(B, P, FD).astype(dt)
        a = a.reshape(N_CORES, IMGS_PER_CORE, P, FD).transpose(0, 2, 1, 3)
        return np.ascontiguousarray(a).reshape(N_CORES, P, TOT_FD)

    xb = pack(cancer_logits, bf)
    pb = pack(prostate_mask, f8)
    nb = pack(needle_mask, f8)
    pnx = np.empty((N_CORES, P, 4 * TOT_FD), dtype=np.uint8)
    xu8 = xb.view(np.uint8)  # [CORE, P, 2*TOT_FD]
    for off, cfd in zip(_CHUNK_OFFS, CHUNK_FDS):
        o4 = 4 * off
        pnx[:, :, o4 : o4 + cfd] = pb[:, :, off : off + cfd].view(np.uint8)
        pnx[:, :, o4 + cfd : o4 + 2 * cfd] = nb[:, :, off : off + cfd].view(
            np.uint8
        )
        pnx[:, :, o4 + 2 * cfd : o4 + 4 * cfd] = xu8[
            :, :, 2 * off : 2 * off + 2 * cfd
        ]
    return [{"pnx8": pnx[c]} for c in range(N_CORES)]


def _combine(results, label):
    y = np.asarray(label, dtype=np.float64).reshape(B)
    ln2 = float(np.log(2.0))
    n_core = IMGS_PER_CORE * N_PER_IMG
    num = 0.0
    cnt = 0.0
    for c in range(N_CORES):
        red = np.asarray(results[c]["red"], dtype=np.float64).reshape(4 * 512)
        sxm = red.reshape(IMGS_PER_CORE, 512).sum(axis=1)
        cols = np.asarray(results[c]["cols"], dtype=np.float64)
        count = cols[:, :N_CHUNKS].sum()
        lns = cols[:, N_CHUNKS:].sum()
        sp_masked = lns - (n_core - count) * ln2
        y_i = y[c * IMGS_PER_CORE : (c + 1) * IMGS_PER_CORE]
        num += sp_masked - (y_i * sxm).sum()
        cnt += count
    return np.float32(num / max(cnt, 1.0))


def kernel(cancer_logits, label, prostate_mask, needle_mask):
    nc = _get_nc()
    in_maps = _make_in_maps(cancer_logits, prostate_mask, needle_mask)
    res = run_bass_kernel_spmd(nc, in_maps, core_ids=list(range(N_CORES)))
    return _combine(res.results, label)
